# revision 61
# baseline (speedup 1.0000x reference)
"""
Trainium2 distributed kernel for nn_AdaptiveGraph (gnn_message_passing).

Reference (N=8192, IN=256, H=128):
    Z = X @ W.T                      # [N, H]
    A = relu(Z @ Z.T)                # [N, N]
    A = A / (A.sum(-1, keepdims) + 1e-6)
    out = A @ Z                      # [N, H]
    returns (out, A)

Strategy: 1D row-parallel over 8 NeuronCores. Core i owns rows
[i*1024, (i+1)*1024). Instead of an on-device all-gather of Z, every core
receives the full X^T (host-side transpose+bf16 cast is part of the input
sharding/layout prep) and computes Z^T_all itself with a few matmuls —
cheaper and simpler than the collective at this size.

Per core:
  prep:    Z^T_all [128, 8192], Z^T_loc [128, 1024], Z_orig blocks [128, 8192]
  phase A: per 128-row tile: S = Z_loc_tile @ Z_all^T via PE (K=h=128);
           ACT relu + accum_out -> relu(S) bf16 + rowsums;
           DVE scale by 1/(rowsum+1e-6) -> A f32 -> DMA (memory roofline path)
  phase B: per 128-col chunk: recompute S^T via PE; relu-cast (split ACT/DVE)
           -> bf16; accumulate out^T += Z_orig_chunk.T-contraction in PSUM
  epilog:  transpose out^T 128x128 blocks on PE, scale by s, DMA out
"""

import os
import numpy as np
import ml_dtypes

import concourse.bass as bass
import concourse.tile as tile
from concourse import bacc, mybir
from concourse.bass import ts, ds
from concourse.bass_utils import run_bass_kernel_spmd
from concourse.masks import make_identity

N = 8192
IN_DIM = 256
HIDDEN = 128
NCORES = 8
L = N // NCORES          # rows per core = 1024
RT = L // 128            # row tiles per core = 8
MC = N // 128            # m chunks = 64
NG = N // 1024           # 1024-wide groups = 8

F32 = mybir.dt.float32
BF16 = mybir.dt.bfloat16
RELU = mybir.ActivationFunctionType.Relu

_NC_CACHE = {}


def build_kernel(profile_friendly: bool = False):
    nc = bacc.Bacc("TRN2", target_bir_lowering=False, debug=False)

    xt = nc.dram_tensor("xt", [IN_DIM, N], BF16, kind="ExternalInput")
    xtl = nc.dram_tensor("xtl", [IN_DIM, L], BF16, kind="ExternalInput")
    wt = nc.dram_tensor("wt", [IN_DIM, HIDDEN], BF16, kind="ExternalInput")
    a_out = nc.dram_tensor("a_out", [L, N], F32, kind="ExternalOutput")
    o_out = nc.dram_tensor("o_out", [L, HIDDEN], F32, kind="ExternalOutput")

    with tile.TileContext(nc) as tc:
        _body(tc, xt, xtl, wt, a_out, o_out)

    nc.compile()
    return nc


def _body(tc, xt, xtl, wt, a_out, o_out):
    nc = tc.nc
    import contextlib

    ctx = contextlib.ExitStack()
    with ctx:
        # ---------------- persistent pools ----------------
        persist = ctx.enter_context(tc.tile_pool(name="persist", bufs=1))
        relus_pool = ctx.enter_context(tc.tile_pool(name="relus", bufs=3))
        relust_pool = ctx.enter_context(tc.tile_pool(name="relust", bufs=8))
        aout_pool = ctx.enter_context(tc.tile_pool(name="aout", bufs=6))
        small = ctx.enter_context(tc.tile_pool(name="small", bufs=2))
        spool = ctx.enter_context(tc.tile_pool(name="srecip", bufs=RT))

        # ---------------- load inputs ----------------
        # W^T as two K-chunks side by side: wt_sb[:, c*128:(c+1)*128] = W^T[c*128:(c+1)*128, :]
        wt_sb = persist.tile([128, 2 * HIDDEN], BF16)
        for c in range(2):
            nc.sync.dma_start(wt_sb[:, ts(c, HIDDEN)], wt[ts(c, 128), :])

        # X^T local columns first (small, unblocks Z^T_local immediately)
        xtl_sb = persist.tile([128, 2 * L], BF16)
        for c in range(2):
            nc.sync.dma_start(xtl_sb[:, ts(c, L)], xtl[ts(c, 128), :])

        # X^T full, two partition-tiles of [128, N]; split DMAs (interleaved
        # across the two K-halves) so dependent compute starts early
        xt_sb = [persist.tile([128, N], BF16, tag=f"xt{c}", name=f"xt_sb{c}")
                 for c in range(2)]
        for q in range(4):
            for c in range(2):
                nc.sync.dma_start(xt_sb[c][:, ts(q, N // 4)],
                                  xt[ts(c, 128), ts(q, N // 4)])

        # identity for PE transposes
        ident = persist.tile([128, 128], BF16)
        make_identity(nc, ident[:])

        # per-partition 0.0 scalar (walrus requires a pointer scalar when a
        # tensor_scalar carries an accumulator output)
        zeros_s = persist.tile([128, 1], F32)
        nc.gpsimd.memset(zeros_s[:], 0.0)

        # ---------------- prep: Z tensors (zt/ztl only; zo is built inline) -------
        zt = persist.tile([128, N], BF16)
        ztl = persist.tile([128, L], BF16)
        zo = persist.tile([128, N], BF16)

        with tc.tile_pool(name="ps_prep", bufs=2, space="PSUM") as ps_prep:
            # Z^T_local [h=128, L] bf16 (xtl lands first)
            for g in range(L // 512):
                ps = ps_prep.tile([128, 512], F32, tag="zt_ps")
                nc.tensor.matmul(ps[:], wt_sb[:, ts(0, 128)], xtl_sb[:, ds(g * 512, 512)],
                                 start=True, stop=False)
                nc.tensor.matmul(ps[:], wt_sb[:, ts(1, 128)],
                                 xtl_sb[:, ds(L + g * 512, 512)],
                                 start=False, stop=True)
                nc.vector.tensor_copy(ztl[:, ts(g, 512)], ps[:])

            # Z^T_all and Z-original blocks, interleaved in xt-quarter arrival
            # order so the PE starts (and warms) as DMAs land. zo done here
            # keeps phase B's inner loop tight.
            for q in range(4):
                for g in range(q * 4, (q + 1) * 4):
                    ps = ps_prep.tile([128, 512], F32, tag="zt_ps")
                    nc.tensor.matmul(ps[:], wt_sb[:, ts(0, 128)],
                                     xt_sb[0][:, ts(g, 512)], start=True, stop=False)
                    nc.tensor.matmul(ps[:], wt_sb[:, ts(1, 128)],
                                     xt_sb[1][:, ts(g, 512)], start=False, stop=True)
                    if g % 2 == 0:
                        nc.scalar.activation(zt[:, ts(g, 512)], ps[:],
                                             mybir.ActivationFunctionType.Copy)
                    else:
                        nc.vector.tensor_copy(zt[:, ts(g, 512)], ps[:])
                for mq in range(q * 4, (q + 1) * 4):
                    ps = ps_prep.tile([128, 512], F32, tag="zo_ps")
                    for j in range(4):
                        b = mq * 4 + j
                        nc.tensor.matmul(ps[:, ts(j, 128)], xt_sb[0][:, ts(b, 128)],
                                         wt_sb[:, ts(0, 128)], start=True, stop=False)
                        nc.tensor.matmul(ps[:, ts(j, 128)], xt_sb[1][:, ts(b, 128)],
                                         wt_sb[:, ts(1, 128)], start=False, stop=True)
                    if mq % 2 == 0:
                        nc.scalar.activation(zo[:, ds(mq * 512, 512)], ps[:],
                                             mybir.ActivationFunctionType.Copy)
                    else:
                        nc.vector.tensor_copy(zo[:, ds(mq * 512, 512)], ps[:])

        ps_a = ctx.enter_context(tc.tile_pool(name="ps_a", bufs=2, space="PSUM"))
        ps_b = ctx.enter_context(tc.tile_pool(name="ps_b", bufs=2, space="PSUM"))
        ps_acc = ctx.enter_context(tc.tile_pool(name="ps_acc", bufs=1, space="PSUM"))

        # Greedy load-balancing of elementwise work across ACT and DVE.
        # Costs in ~us per op, from HW traces.
        debt = {"ACT": 0.0, "DVE": 0.0}

        def pick(cost_act, cost_dve):
            if debt["ACT"] + cost_act <= debt["DVE"] + cost_dve:
                debt["ACT"] += cost_act
                return "ACT"
            debt["DVE"] += cost_dve
            return "DVE"

        # ------- interleaved phases -------------------------------------------
        # A-step k (64 total): row-tile rt = k//8, group g = k%8:
        #   S chunk [128,1024] -> ACT relu + accum -> bf16 relus; at g==7 compute
        #   s = 1/(rowsum+eps); then DVE scale + DMA A rows.
        # B-step k (64 total): m-chunk mc = k:
        #   zo block (4-packed MMs + one [128,512] cast every 4th step),
        #   S^T chunk -> relu-cast (ACT/DVE split) -> out^T accumulation.
        s_tiles = []
        relus_tiles = {}
        acc_tiles = {}
        outT_ps = ps_acc.tile([128, L], F32)

        def relu_cast(dst, src, accum, cost_act, cost_dve):
            if pick(cost_act, cost_dve) == "DVE":
                nc.vector.tensor_scalar(dst, src, zeros_s[:], zeros_s[:],
                                        op0=mybir.AluOpType.max,
                                        op1=mybir.AluOpType.add, accum_out=accum)
            else:
                nc.scalar.activation(dst, src, RELU, accum_out=accum)

        def emit_a_mm(k):
            rt, g = divmod(k, NG)
            if g == 0:
                relus_tiles[rt] = relus_pool.tile([128, N], BF16, tag="relus",
                                                  name=f"relus{rt}")
                acc_tiles[rt] = small.tile([128, NG], F32, tag="acc", name=f"acc{rt}")
            relus_t, acc = relus_tiles[rt], acc_tiles[rt]
            ps = ps_a.tile([128, 1024], F32, tag="sa_ps")
            nc.tensor.matmul(ps[:, 0:512], ztl[:, ts(rt, 128)],
                             zt[:, ds(g * 1024, 512)], start=True, stop=True)
            nc.tensor.matmul(ps[:, 512:1024], ztl[:, ts(rt, 128)],
                             zt[:, ds(g * 1024 + 512, 512)], start=True, stop=True)
            relu_cast(relus_t[:, ts(g, 1024)], ps[:], acc[:, ds(g, 1)],
                      cost_act=1.29, cost_dve=1.40)
            if g == NG - 1:
                ssum = small.tile([128, 1], F32, tag="ssum")
                nc.vector.reduce_sum(ssum[:], acc[:], axis=mybir.AxisListType.X)
                s_rt = spool.tile([128, 1], F32, tag="s", name=f"s{rt}")
                nc.vector.tensor_scalar_add(ssum[:], ssum[:], 1e-6)
                nc.vector.reciprocal(s_rt[:], ssum[:])
                s_tiles.append(s_rt)
                # scale in bf16 (DVE 4x read / 2x write) and let the SWDGE DMA
                # cast bf16 -> f32 on the way to HBM (halves SBUF-port reads)
                for gg in range(NG // 4):
                    a_t = aout_pool.tile([128, 4096], BF16, tag="a")
                    if pick(3.70, 1.15) == "DVE":
                        nc.vector.tensor_scalar_mul(a_t[:], relus_t[:, ts(gg, 4096)],
                                                    s_rt[:])
                    else:
                        nc.scalar.activation(a_t[:], relus_t[:, ts(gg, 4096)],
                                             mybir.ActivationFunctionType.Copy,
                                             scale=s_rt[:])
                    nc.gpsimd.dma_start(a_out[ts(rt, 128), ds(gg * 4096, 4096)],
                                        a_t[:])


        # out^T accumulation runs DELAY chunks behind the S^T/relu stream so the
        # PE never stalls in-FIFO waiting for a relust that was just produced.
        DELAY = 2
        relust_q = []

        def emit_outT(mc, relust_t):
            first = mc == 0
            last = mc == MC - 1
            for h in range(2):
                nc.tensor.matmul(outT_ps[:, ts(h, 512)], zo[:, ts(mc, 128)],
                                 relust_t[:, ts(h, 512)],
                                 start=first, stop=last, skip_group_check=True)

        def emit_b_mm(mc):
            relust_t = relust_pool.tile([128, 1024], BF16, tag="relust")
            for h in range(2):
                ps = ps_b.tile([128, 512], F32, tag="st_ps", bufs=2)
                nc.tensor.matmul(ps[:], zt[:, ts(mc, 128)], ztl[:, ts(h, 512)],
                                 start=True, stop=True)
                relu_cast(relust_t[:, ts(h, 512)], ps[:], None,
                          cost_act=0.62, cost_dve=0.75)
            relust_q.append((mc, relust_t))
            if len(relust_q) > DELAY:
                emit_outT(*relust_q.pop(0))

        # front-load phase A 2:1 early (its DMA stream is the roofline)
        a_next = 0
        for k in range(MC):
            n_a = 2 if k < 8 else 1
            for _ in range(n_a):
                if a_next < MC:
                    emit_a_mm(a_next)
                    a_next += 1
            emit_b_mm(k)

        # drain the deferred out^T accumulations
        while relust_q:
            emit_outT(*relust_q.pop(0))

        # ---------------- epilogue: out = s * transpose(out^T) ----------------
        # All 8 tiles scale into one staging tile and leave in a single DMA
        # (8 separate 64 KB DMAs serialized on ~2us fixed costs before).
        outT_sb = persist.tile([128, L], BF16)
        nc.vector.tensor_copy(outT_sb[:], outT_ps[:])
        o_all = persist.tile([128, L], F32)
        for rt in range(RT):
            # reuse the phase-A PSUM slot (phase A is long done by now)
            tp = ps_a.tile([128, 128], BF16, tag="sa_ps", name="tp")
            nc.tensor.transpose(tp[:], outT_sb[:, ts(rt, 128)], ident[:])
            nc.vector.tensor_scalar_mul(o_all[:, ts(rt, 128)], tp[:],
                                        s_tiles[rt][:])
        # o_all[p, rt*128+c] = out[rt*128+p, c]
        nc.sync.dma_start(o_out.ap().rearrange("(r p) c -> p r c", p=128),
                          o_all[:].rearrange("p (r c) -> p r c", c=HIDDEN))


def _get_nc():
    if "nc" not in _NC_CACHE:
        _NC_CACHE["nc"] = build_kernel()
    return _NC_CACHE["nc"]


def kernel(X: np.ndarray, W: np.ndarray):
    X = np.asarray(X, dtype=np.float32)
    W = np.asarray(W, dtype=np.float32)
    assert X.shape == (N, IN_DIM) and W.shape == (HIDDEN, IN_DIM)

    bf = ml_dtypes.bfloat16
    xt_np = np.ascontiguousarray(X.T).astype(bf)          # [256, 8192]
    wt_np = np.ascontiguousarray(W.T).astype(bf)          # [256, 128]

    in_maps = []
    for i in range(NCORES):
        in_maps.append({
            "xt": xt_np,
            "xtl": np.ascontiguousarray(xt_np[:, i * L:(i + 1) * L]),
            "wt": wt_np,
        })

    nc = _get_nc()
    res = run_bass_kernel_spmd(nc, in_maps, core_ids=list(range(NCORES)))
    out = np.concatenate([res.results[i]["o_out"] for i in range(NCORES)], axis=0)
    A = np.concatenate([res.results[i]["a_out"] for i in range(NCORES)], axis=0)
    return (np.asarray(out, dtype=np.float32), np.asarray(A, dtype=np.float32))


if __name__ == "__main__":
    rng = np.random.default_rng(0)
    X = rng.standard_normal((N, IN_DIM), dtype=np.float32)
    W = (rng.standard_normal((HIDDEN, IN_DIM), dtype=np.float32) / np.sqrt(IN_DIM)).astype(np.float32)
    out, A = kernel(X, W)
    print("out", out.shape, out.dtype, "A", A.shape, A.dtype)


# revision 62
# speedup vs baseline: 1.1479x; 1.1479x over previous
"""
Trainium2 distributed kernel for nn_AdaptiveGraph (gnn_message_passing).

Reference (N=8192, IN=256, H=128):
    Z = X @ W.T                      # [N, H]
    A = relu(Z @ Z.T)                # [N, N]
    A = A / (A.sum(-1, keepdims) + 1e-6)
    out = A @ Z                      # [N, H]
    returns (out, A)

Strategy: 1D row-parallel over 8 NeuronCores. Core i owns rows
[i*1024, (i+1)*1024). Instead of an on-device all-gather of Z, every core
receives the full X^T (host-side transpose+bf16 cast is part of the input
sharding/layout prep) and computes Z^T_all itself with a few matmuls —
cheaper and simpler than the collective at this size.

Per core:
  prep:    Z^T_all [128, 8192], Z^T_loc [128, 1024], Z_orig blocks [128, 8192]
  phase A: per 128-row tile: S = Z_loc_tile @ Z_all^T via PE (K=h=128);
           ACT relu + accum_out -> relu(S) bf16 + rowsums;
           DVE scale by 1/(rowsum+1e-6) -> A f32 -> DMA (memory roofline path)
  phase B: per 128-col chunk: recompute S^T via PE; relu-cast (split ACT/DVE)
           -> bf16; accumulate out^T += Z_orig_chunk.T-contraction in PSUM
  epilog:  transpose out^T 128x128 blocks on PE, scale by s, DMA out
"""

import os
import numpy as np
import ml_dtypes

import concourse.bass as bass
import concourse.tile as tile
from concourse import bacc, mybir
from concourse.bass import ts, ds
from concourse.bass_utils import run_bass_kernel_spmd
from concourse.masks import make_identity

N = 8192
IN_DIM = 256
HIDDEN = 128
NCORES = 8
L = N // NCORES          # rows per core = 1024
RT = L // 128            # row tiles per core = 8
MC = N // 128            # m chunks = 64
NG = N // 1024           # 1024-wide groups = 8

F32 = mybir.dt.float32
BF16 = mybir.dt.bfloat16
RELU = mybir.ActivationFunctionType.Relu

_NC_CACHE = {}


def build_kernel(profile_friendly: bool = False):
    nc = bacc.Bacc("TRN2", target_bir_lowering=False, debug=False)

    xt = nc.dram_tensor("xt", [IN_DIM, N], BF16, kind="ExternalInput")
    xtl = nc.dram_tensor("xtl", [IN_DIM, L], BF16, kind="ExternalInput")
    wt = nc.dram_tensor("wt", [IN_DIM, HIDDEN], BF16, kind="ExternalInput")
    a_out = nc.dram_tensor("a_out", [L, N], F32, kind="ExternalOutput")
    o_out = nc.dram_tensor("o_out", [L, HIDDEN], F32, kind="ExternalOutput")

    with tile.TileContext(nc) as tc:
        _body(tc, xt, xtl, wt, a_out, o_out)

    nc.compile()
    return nc


def _body(tc, xt, xtl, wt, a_out, o_out):
    nc = tc.nc
    import contextlib

    ctx = contextlib.ExitStack()
    with ctx:
        # ---------------- persistent pools ----------------
        persist = ctx.enter_context(tc.tile_pool(name="persist", bufs=1))
        relus_pool = ctx.enter_context(tc.tile_pool(name="relus", bufs=3))
        relust_pool = ctx.enter_context(tc.tile_pool(name="relust", bufs=8))
        aout_pool = ctx.enter_context(tc.tile_pool(name="aout", bufs=6))
        small = ctx.enter_context(tc.tile_pool(name="small", bufs=2))
        spool = ctx.enter_context(tc.tile_pool(name="srecip", bufs=RT))

        # ---------------- load inputs ----------------
        # W^T as two K-chunks side by side: wt_sb[:, c*128:(c+1)*128] = W^T[c*128:(c+1)*128, :]
        wt_sb = persist.tile([128, 2 * HIDDEN], BF16)
        for c in range(2):
            nc.sync.dma_start(wt_sb[:, ts(c, HIDDEN)], wt[ts(c, 128), :])

        # X^T local columns first (small, unblocks Z^T_local immediately)
        xtl_sb = persist.tile([128, 2 * L], BF16)
        for c in range(2):
            nc.sync.dma_start(xtl_sb[:, ts(c, L)], xtl[ts(c, 128), :])

        # X^T full, two partition-tiles of [128, N]; split DMAs (interleaved
        # across the two K-halves) so dependent compute starts early
        xt_sb = [persist.tile([128, N], BF16, tag=f"xt{c}", name=f"xt_sb{c}")
                 for c in range(2)]
        for q in range(4):
            for c in range(2):
                nc.sync.dma_start(xt_sb[c][:, ts(q, N // 4)],
                                  xt[ts(c, 128), ts(q, N // 4)])

        # identity for PE transposes
        ident = persist.tile([128, 128], BF16)
        make_identity(nc, ident[:])

        # per-partition 0.0 scalar (walrus requires a pointer scalar when a
        # tensor_scalar carries an accumulator output)
        zeros_s = persist.tile([128, 1], F32)
        nc.gpsimd.memset(zeros_s[:], 0.0)

        # ---------------- prep: Z tensors (zt/ztl only; zo is built inline) -------
        zt = persist.tile([128, N], BF16)
        ztl = persist.tile([128, L], BF16)
        zo = persist.tile([128, N], BF16)

        with tc.tile_pool(name="ps_prep", bufs=2, space="PSUM") as ps_prep:
            # Z^T_local [h=128, L] bf16 (xtl lands first)
            for g in range(L // 512):
                ps = ps_prep.tile([128, 512], F32, tag="zt_ps")
                nc.tensor.matmul(ps[:], wt_sb[:, ts(0, 128)], xtl_sb[:, ds(g * 512, 512)],
                                 start=True, stop=False)
                nc.tensor.matmul(ps[:], wt_sb[:, ts(1, 128)],
                                 xtl_sb[:, ds(L + g * 512, 512)],
                                 start=False, stop=True)
                nc.vector.tensor_copy(ztl[:, ts(g, 512)], ps[:])

            # Z^T_all and Z-original blocks, interleaved in xt-quarter arrival
            # order so the PE starts (and warms) as DMAs land. zo done here
            # keeps phase B's inner loop tight.
            for q in range(4):
                for g in range(q * 4, (q + 1) * 4):
                    ps = ps_prep.tile([128, 512], F32, tag="zt_ps")
                    nc.tensor.matmul(ps[:], wt_sb[:, ts(0, 128)],
                                     xt_sb[0][:, ts(g, 512)], start=True, stop=False)
                    nc.tensor.matmul(ps[:], wt_sb[:, ts(1, 128)],
                                     xt_sb[1][:, ts(g, 512)], start=False, stop=True)
                    if g % 2 == 0:
                        nc.scalar.activation(zt[:, ts(g, 512)], ps[:],
                                             mybir.ActivationFunctionType.Copy)
                    else:
                        nc.vector.tensor_copy(zt[:, ts(g, 512)], ps[:])
                for mq in range(q * 4, (q + 1) * 4):
                    ps = ps_prep.tile([128, 512], F32, tag="zo_ps")
                    for j in range(4):
                        b = mq * 4 + j
                        nc.tensor.matmul(ps[:, ts(j, 128)], xt_sb[0][:, ts(b, 128)],
                                         wt_sb[:, ts(0, 128)], start=True, stop=False)
                        nc.tensor.matmul(ps[:, ts(j, 128)], xt_sb[1][:, ts(b, 128)],
                                         wt_sb[:, ts(1, 128)], start=False, stop=True)
                    if mq % 2 == 0:
                        nc.scalar.activation(zo[:, ds(mq * 512, 512)], ps[:],
                                             mybir.ActivationFunctionType.Copy)
                    else:
                        nc.vector.tensor_copy(zo[:, ds(mq * 512, 512)], ps[:])

        ps_a = ctx.enter_context(tc.tile_pool(name="ps_a", bufs=2, space="PSUM"))
        ps_b = ctx.enter_context(tc.tile_pool(name="ps_b", bufs=2, space="PSUM"))
        ps_acc = ctx.enter_context(tc.tile_pool(name="ps_acc", bufs=1, space="PSUM"))

        # Greedy load-balancing of elementwise work across ACT and DVE.
        # Costs in ~us per op, from HW traces.
        debt = {"ACT": 0.0, "DVE": 0.0}

        def pick(cost_act, cost_dve):
            if debt["ACT"] + cost_act <= debt["DVE"] + cost_dve:
                debt["ACT"] += cost_act
                return "ACT"
            debt["DVE"] += cost_dve
            return "DVE"

        # ------- interleaved phases -------------------------------------------
        # A-step k (64 total): row-tile rt = k//8, group g = k%8:
        #   S chunk [128,1024] -> ACT relu + accum -> bf16 relus; at g==7 compute
        #   s = 1/(rowsum+eps); then DVE scale + DMA A rows.
        # B-step k (64 total): m-chunk mc = k:
        #   zo block (4-packed MMs + one [128,512] cast every 4th step),
        #   S^T chunk -> relu-cast (ACT/DVE split) -> out^T accumulation.
        s_tiles = []
        relus_tiles = {}
        acc_tiles = {}
        outT_ps = ps_acc.tile([128, L], F32)

        def relu_cast(dst, src, accum, cost_act, cost_dve):
            if pick(cost_act, cost_dve) == "DVE":
                nc.vector.tensor_scalar(dst, src, zeros_s[:], zeros_s[:],
                                        op0=mybir.AluOpType.max,
                                        op1=mybir.AluOpType.add, accum_out=accum)
            else:
                nc.scalar.activation(dst, src, RELU, accum_out=accum)

        def emit_a_mm(k):
            rt, g = divmod(k, NG)
            if g == 0:
                relus_tiles[rt] = relus_pool.tile([128, N], BF16, tag="relus",
                                                  name=f"relus{rt}")
                acc_tiles[rt] = small.tile([128, NG], F32, tag="acc", name=f"acc{rt}")
            relus_t, acc = relus_tiles[rt], acc_tiles[rt]
            ps = ps_a.tile([128, 1024], F32, tag="sa_ps")
            nc.tensor.matmul(ps[:, 0:512], ztl[:, ts(rt, 128)],
                             zt[:, ds(g * 1024, 512)], start=True, stop=True)
            nc.tensor.matmul(ps[:, 512:1024], ztl[:, ts(rt, 128)],
                             zt[:, ds(g * 1024 + 512, 512)], start=True, stop=True)
            relu_cast(relus_t[:, ts(g, 1024)], ps[:], acc[:, ds(g, 1)],
                      cost_act=1.29, cost_dve=1.40)
            if g == NG - 1:
                ssum = small.tile([128, 1], F32, tag="ssum")
                nc.vector.reduce_sum(ssum[:], acc[:], axis=mybir.AxisListType.X)
                s_rt = spool.tile([128, 1], F32, tag="s", name=f"s{rt}")
                nc.vector.tensor_scalar_add(ssum[:], ssum[:], 1e-6)
                nc.vector.reciprocal(s_rt[:], ssum[:])
                s_tiles.append(s_rt)
                # scale in bf16 (DVE 4x read / 2x write) and let the SWDGE DMA
                # cast bf16 -> f32 on the way to HBM (halves SBUF-port reads)
                for gg in range(NG // 4):
                    a_t = aout_pool.tile([128, 4096], BF16, tag="a")
                    if pick(3.70, 1.15) == "DVE":
                        nc.vector.tensor_scalar_mul(a_t[:], relus_t[:, ts(gg, 4096)],
                                                    s_rt[:])
                    else:
                        nc.scalar.activation(a_t[:], relus_t[:, ts(gg, 4096)],
                                             mybir.ActivationFunctionType.Copy,
                                             scale=s_rt[:])
                    nc.gpsimd.dma_start(a_out[ts(rt, 128), ds(gg * 4096, 4096)],
                                        a_t[:])


        # out^T accumulation runs DELAY chunks behind the S^T/relu stream so the
        # PE never stalls in-FIFO waiting for a relust that was just produced.
        DELAY = 2
        relust_q = []

        def emit_outT(mc, relust_t):
            first = mc == 0
            last = mc == MC - 1
            for h in range(2):
                nc.tensor.matmul(outT_ps[:, ts(h, 512)], zo[:, ts(mc, 128)],
                                 relust_t[:, ts(h, 512)],
                                 start=first, stop=last, skip_group_check=True)

        def emit_b_mm(mc):
            relust_t = relust_pool.tile([128, 1024], BF16, tag="relust")
            for h in range(2):
                ps = ps_b.tile([128, 512], F32, tag="st_ps", bufs=2)
                nc.tensor.matmul(ps[:], zt[:, ts(mc, 128)], ztl[:, ts(h, 512)],
                                 start=True, stop=True)
                relu_cast(relust_t[:, ts(h, 512)], ps[:], None,
                          cost_act=0.62, cost_dve=0.75)
            relust_q.append((mc, relust_t))
            if len(relust_q) > DELAY:
                emit_outT(*relust_q.pop(0))

        # front-load phase A 2:1 early (its DMA stream is the roofline)
        a_next = 0
        for k in range(MC):
            n_a = 2 if k < 16 else 1
            for _ in range(n_a):
                if a_next < MC:
                    emit_a_mm(a_next)
                    a_next += 1
            emit_b_mm(k)

        # drain the deferred out^T accumulations
        while relust_q:
            emit_outT(*relust_q.pop(0))

        # ---------------- epilogue: out = s * transpose(out^T) ----------------
        outT_sb = persist.tile([128, L], BF16)
        nc.vector.tensor_copy(outT_sb[:], outT_ps[:])
        for rt in range(RT):
            # reuse the phase-A PSUM slot (phase A is long done by now)
            tp = ps_a.tile([128, 128], BF16, tag="sa_ps", name="tp")
            nc.tensor.transpose(tp[:], outT_sb[:, ts(rt, 128)], ident[:])
            o_t = small.tile([128, HIDDEN], F32, tag="o", bufs=4)
            nc.vector.tensor_scalar_mul(o_t[:], tp[:], s_tiles[rt][:])
            # alternate the two HWDGE rings so per-DMA fixed costs overlap
            eng = nc.sync if rt % 2 == 0 else nc.scalar
            eng.dma_start(o_out[ts(rt, 128), :], o_t[:])


def _get_nc():
    if "nc" not in _NC_CACHE:
        _NC_CACHE["nc"] = build_kernel()
    return _NC_CACHE["nc"]


def kernel(X: np.ndarray, W: np.ndarray):
    X = np.asarray(X, dtype=np.float32)
    W = np.asarray(W, dtype=np.float32)
    assert X.shape == (N, IN_DIM) and W.shape == (HIDDEN, IN_DIM)

    bf = ml_dtypes.bfloat16
    xt_np = np.ascontiguousarray(X.T).astype(bf)          # [256, 8192]
    wt_np = np.ascontiguousarray(W.T).astype(bf)          # [256, 128]

    in_maps = []
    for i in range(NCORES):
        in_maps.append({
            "xt": xt_np,
            "xtl": np.ascontiguousarray(xt_np[:, i * L:(i + 1) * L]),
            "wt": wt_np,
        })

    nc = _get_nc()
    res = run_bass_kernel_spmd(nc, in_maps, core_ids=list(range(NCORES)))
    out = np.concatenate([res.results[i]["o_out"] for i in range(NCORES)], axis=0)
    A = np.concatenate([res.results[i]["a_out"] for i in range(NCORES)], axis=0)
    return (np.asarray(out, dtype=np.float32), np.asarray(A, dtype=np.float32))


if __name__ == "__main__":
    rng = np.random.default_rng(0)
    X = rng.standard_normal((N, IN_DIM), dtype=np.float32)
    W = (rng.standard_normal((HIDDEN, IN_DIM), dtype=np.float32) / np.sqrt(IN_DIM)).astype(np.float32)
    out, A = kernel(X, W)
    print("out", out.shape, out.dtype, "A", A.shape, A.dtype)
